# revision 20
# baseline (speedup 1.0000x reference)
"""MultiHeadAttention (GQA + symmetric ALiBi) on 8 trn2 NeuronCores.

Sharding: core = (batch n in {0,1}) x (head-group g in {0..3}).
Each core handles 4 query heads (one GQA pair of kv heads) for one batch:
  - QKV projections in fp32 (x^T streamed, weights column-sliced per group)
  - S^T = K^T.T @ Q^T per (head, k-chunk) in fp32
  - softmax: exp((S + alibi)/8) = exp(S/8) * exp(alibi/8); the alibi factor
    is a host-precomputed bf16 table indexed by (i - u + 1920); no max-sub
    needed (exp args are O(1))
  - row sums via a ones-column appended to V (bf16 PV matmul, fp32 accum)
  - normalization: reciprocal of sums broadcast across partitions via a
    rank-1 matmul with a ones vector
  - output projection partial = attnT @ Wo^T slice + bo/4 (bias folded in as
    a K=1 matmul); host sums the 4 group partials per batch.
"""
import sys

sys.path.insert(0, "/opt/trn_rl_repo")
import numpy as np
import ml_dtypes

import concourse.bass as bass
import concourse.mybir as mybir
from concourse import bacc
from concourse.tile import TileContext
from concourse.masks import make_identity
from concourse.bass_utils import run_bass_kernel_spmd


def _register_ntff_hook_module():
    # bass_utils imports antenv.axon_hooks for trace=True under axon; this
    # image's antenv lacks it, so register our shim in sys.modules.
    import importlib.util

    if "antenv.axon_hooks" in sys.modules:
        return
    path = "/opt/trn_rl_repo/antenv/axon_hooks.py"
    try:
        spec = importlib.util.spec_from_file_location("antenv.axon_hooks", path)
        mod = importlib.util.module_from_spec(spec)
        spec.loader.exec_module(mod)
        sys.modules["antenv.axon_hooks"] = mod
    except Exception:
        pass


_register_ntff_hook_module()

S = 2048
E = 1024
D = 64
TW = 3968  # alibi exp-table width: u = j - k0 + 1920 in [0, 3968)
F32 = mybir.dt.float32
BF16 = mybir.dt.bfloat16

_NC = None
LAST_RESULTS = None


def _build():
    nc = bacc.Bacc("TRN2", target_bir_lowering=False, debug=False, num_devices=8)
    xT = nc.dram_tensor("xT", [E, S], BF16, kind="ExternalInput")
    wqT = nc.dram_tensor("wqT", [E, 256], BF16, kind="ExternalInput")
    wkT = nc.dram_tensor("wkT", [E, 128], BF16, kind="ExternalInput")
    wvT = nc.dram_tensor("wvT", [E, 128], BF16, kind="ExternalInput")
    woT = nc.dram_tensor("woT", [256, E], BF16, kind="ExternalInput")
    bo4 = nc.dram_tensor("bo4", [1, E], BF16, kind="ExternalInput")
    texp = nc.dram_tensor("texp", [4, 128, TW], BF16, kind="ExternalInput")
    outT = nc.dram_tensor("outT", [E, S], F32, kind="ExternalOutput")

    Exp = mybir.ActivationFunctionType.Exp

    with TileContext(nc) as tc:
        with (
            tc.sbuf_pool(name="const", bufs=1) as const,
            tc.sbuf_pool(name="qkv", bufs=1) as qkv,
            tc.sbuf_pool(name="pp", bufs=3) as pp,
            tc.sbuf_pool(name="norm", bufs=1) as norm,
        ):
            # ---- constants / weights
            wq_sb = const.tile([128, 8 * 256], BF16)
            nc.sync.dma_start(
                out=wq_sb.rearrange("p (c m) -> p c m", m=256),
                in_=wqT.rearrange("(c p) m -> p c m", p=128),
            )
            wk_sb = const.tile([128, 8 * 128], BF16)
            nc.sync.dma_start(
                out=wk_sb.rearrange("p (c m) -> p c m", m=128),
                in_=wkT.rearrange("(c p) m -> p c m", p=128),
            )
            wv_sb = const.tile([128, 8 * 128], BF16)
            nc.sync.dma_start(
                out=wv_sb.rearrange("p (c m) -> p c m", m=128),
                in_=wvT.rearrange("(c p) m -> p c m", p=128),
            )
            wo_sb = const.tile([128, 2 * 1024], BF16)
            nc.sync.dma_start(
                out=wo_sb.rearrange("p (c m) -> p c m", m=1024),
                in_=woT.rearrange("(c p) m -> p c m", p=128),
            )
            bo_sb = const.tile([1, E], BF16)
            nc.sync.dma_start(out=bo_sb, in_=bo4[:, :])
            tex_sb = const.tile([128, 4 * TW], BF16)
            for h in range(4):
                nc.sync.dma_start(out=tex_sb[:, h * TW : (h + 1) * TW], in_=texp[h])
            ones_sb = const.tile([1, 512], BF16)
            nc.vector.memset(ones_sb, 1.0)
            ident = const.tile([128, 128], BF16)
            make_identity(nc, ident)

            # QT/KT zero-padded to K=128 and VS to 128 cols: square bf16
            # stationaries keep the PE weight path fast (FWL), and the zero
            # rows/cols contribute nothing to the results.
            QT = [qkv.tile([128, S], BF16, name=f"qt{h}") for h in range(4)]
            KT = [qkv.tile([128, S], BF16, name=f"kt{k}") for k in range(2)]
            vt_sb = qkv.tile([128, S], BF16)
            VS = [qkv.tile([128, 16 * 128], BF16, name=f"vs{k}") for k in range(2)]
            AT = [qkv.tile([128, S], BF16, name=f"at{c}") for c in range(2)]
            for t in QT + KT:
                nc.vector.memset(t, 0.0)

            # ---- Phase A: QKV projections (fp32)
            with (
                tc.sbuf_pool(name="xp", bufs=3) as xp,
                tc.psum_pool(name="qkvps", bufs=4) as qp,
            ):
                for qh in range(2):  # 1024-wide q halves
                    ps_q0 = qp.tile([128, 1024], F32, tag="qkvps", name="ps_q0")
                    ps_q1 = qp.tile([128, 1024], F32, tag="qkvps", name="ps_q1")
                    ps_k = qp.tile([128, 1024], F32, tag="qkvps", name="ps_k")
                    ps_v = qp.tile([128, 1024], F32, tag="qkvps", name="ps_v")
                    for e in range(8):
                        xt = xp.tile([128, 1024], BF16, tag="x", name="xt")
                        nc.sync.dma_start(
                            out=xt,
                            in_=xT[e * 128 : (e + 1) * 128, qh * 1024 : (qh + 1) * 1024],
                        )
                        st, sp = (e == 0), (e == 7)
                        targets = [
                            (ps_q0, wq_sb[:, e * 256 : e * 256 + 128]),
                            (ps_q1, wq_sb[:, e * 256 + 128 : e * 256 + 256]),
                            (ps_k, wk_sb[:, e * 128 : (e + 1) * 128]),
                            (ps_v, wv_sb[:, e * 128 : (e + 1) * 128]),
                        ]
                        for ps, w in targets:
                            for i in range(2):
                                osl = slice(i * 512, (i + 1) * 512)
                                nc.tensor.matmul(
                                    ps[:, osl], w, xt[:, osl], start=st, stop=sp,
                                )
                    qs = slice(qh * 1024, (qh + 1) * 1024)
                    nc.vector.tensor_copy(vt_sb[:, qs], ps_v)
                    nc.vector.tensor_copy(KT[0][0:64, qs], ps_k[0:64, :])
                    nc.vector.tensor_copy(KT[1][0:64, qs], ps_k[64:128, :])
                    nc.scalar.copy(QT[0][0:64, qs], ps_q0[0:64, :])
                    nc.scalar.copy(QT[1][0:64, qs], ps_q0[64:128, :])
                    nc.scalar.copy(QT[2][0:64, qs], ps_q1[0:64, :])
                    nc.scalar.copy(QT[3][0:64, qs], ps_q1[64:128, :])

            # ---- Phase A2: V^T -> V (PE transposes), append ones column
            for kv in range(2):
                nc.vector.memset(VS[kv], 0.0)
                nc.vector.memset(
                    VS[kv].rearrange("p (c m) -> p c m", m=128)[:, :, 64:65], 1.0
                )
            with tc.psum_pool(name="tpps", bufs=4) as tp:
                for kc in range(16):
                    pt = tp.tile([128, 128], BF16, tag="tp", name="pt")
                    nc.tensor.transpose(pt, vt_sb[:, kc * 128 : (kc + 1) * 128], ident)
                    for kv in range(2):
                        nc.vector.tensor_copy(
                            VS[kv][:, kc * 128 : kc * 128 + 64],
                            pt[:, kv * 64 : (kv + 1) * 64],
                        )

            # ---- Phase C: attention per head
            with (
                tc.psum_pool(name="sps", bufs=2) as spp,
                tc.psum_pool(name="pvps", bufs=1) as pvp,
            ):
                pend = {}  # h -> (pvs, r) awaiting broadcast + final scale

                def head_attention(h):
                    kv = h // 2
                    pv = pvp.tile([128, 2048], F32, tag="pv", name="pv")
                    for kc in range(16):
                        ks = slice(kc * 128, (kc + 1) * 128)
                        # 4 S matmuls back-to-back (one stationary load)
                        ss = [
                            spp.tile([128, 1024], F32, tag="s", name="s"),
                            spp.tile([128, 1024], F32, tag="s", name="s2"),
                        ]
                        for qq in range(4):
                            nc.tensor.matmul(
                                ss[qq // 2][:, (qq % 2) * 512 : (qq % 2 + 1) * 512],
                                KT[kv][:, ks],
                                QT[h][:, qq * 512 : (qq + 1) * 512],
                                start=True, stop=True,
                            )
                        pexp = pp.tile([128, 2048], BF16, tag="pexp", name="pexp")
                        for qh in range(2):
                            nc.scalar.activation(
                                pexp[:, qh * 1024 : (qh + 1) * 1024],
                                ss[qh], Exp, scale=0.125,
                            )
                        p = pp.tile([128, 2048], BF16, tag="p", name="p")
                        u0 = h * TW + 1920 - kc * 128
                        nc.vector.tensor_mul(p, pexp, tex_sb[:, u0 : u0 + 2048])
                        # 4 PV matmuls back-to-back (one stationary load)
                        for qq in range(4):
                            nc.tensor.matmul(
                                pv[:, qq * 512 : (qq + 1) * 512],
                                VS[kv][:, kc * 128 : (kc + 1) * 128],
                                p[:, qq * 512 : (qq + 1) * 512],
                                start=(kc == 0), stop=(kc == 15),
                                skip_group_check=True,
                            )
                    # Move pv out of PSUM at once (DVE + ACT in parallel) and
                    # run the reciprocal chain; the PE broadcast + final scale
                    # are deferred one head so this chain never stalls the PE.
                    pvs = norm.tile([64, 2048], F32, tag="pvs", name="pvs", bufs=2)
                    nc.vector.tensor_copy(pvs, pv[0:64, :])
                    rsum = norm.tile([1, 2048], F32, tag="rsum", name="rsum", bufs=2)
                    nc.scalar.copy(rsum, pv[64:65, :])
                    r128 = norm.tile([128, 16], F32, tag="r128", name="r128", bufs=2)
                    nc.sync.dma_start(out=r128, in_=rsum)
                    rr = norm.tile([128, 16], F32, tag="rr", name="rr", bufs=2)
                    nc.vector.reciprocal(rr, r128)
                    rrb = norm.tile([128, 16], BF16, tag="rrb", name="rrb", bufs=2)
                    nc.vector.tensor_copy(rrb, rr)
                    r = norm.tile([1, 2048], BF16, tag="r", name="r", bufs=2)
                    nc.sync.dma_start(out=r, in_=rrb)
                    pend[h] = (pvs, r)

                def head_normalize(h):
                    pvs, r = pend.pop(h)
                    rbs = norm.tile([64, 2048], F32, tag="rbs", name="rbs", bufs=1)
                    for half in range(2):
                        rb = spp.tile([64, 1024], F32, tag="s", name="rb")
                        for qq in range(2):
                            c0 = half * 1024 + qq * 512
                            nc.tensor.matmul(
                                rb[:, qq * 512 : (qq + 1) * 512],
                                (ones_sb[:, 0:64]),
                                (r[:, c0 : c0 + 512]),
                                start=True, stop=True,
                            )
                        nc.vector.tensor_copy(
                            rbs[:, half * 1024 : (half + 1) * 1024], rb
                        )
                    at = AT[h // 2]
                    r0 = 64 * (h % 2)
                    nc.vector.tensor_mul(at[r0 : r0 + 64, :], pvs, rbs)

                for h in range(4):
                    head_attention(h)
                    if h > 0:
                        head_normalize(h - 1)
                head_normalize(3)

            # ---- Phase D: output projection (+ bias/4 via K=1 matmul)
            with (
                tc.psum_pool(name="ops", bufs=8) as op,
                tc.sbuf_pool(name="osb", bufs=4) as osb,
            ):
                for ec in range(8):
                    os_ = [op.tile([128, 512], F32, tag="o", name="o") for _ in range(4)]
                    for c in range(2):
                        w = wo_sb[:, c * 1024 + ec * 128 : c * 1024 + (ec + 1) * 128]
                        for qc in range(4):
                            qs = slice(qc * 512, (qc + 1) * 512)
                            nc.tensor.matmul(
                                os_[qc], w, AT[c][:, qs],
                                start=(c == 0), stop=False,
                                skip_group_check=True,
                            )
                    wb = bo_sb[:, ec * 128 : (ec + 1) * 128]
                    for qc in range(4):
                        nc.tensor.matmul(
                            os_[qc], wb, ones_sb[:, 0:512],
                            start=False, stop=True,
                            skip_group_check=True,
                        )
                    for qc in range(4):
                        qs = slice(qc * 512, (qc + 1) * 512)
                        o_sb = osb.tile([128, 512], F32, tag="osb", name="o_sb")
                        nc.vector.tensor_copy(o_sb, os_[qc])
                        nc.sync.dma_start(
                            out=outT[ec * 128 : (ec + 1) * 128, qs], in_=o_sb
                        )

    nc.compile()
    return nc


def _texp_tables():
    i = np.arange(128, dtype=np.float64).reshape(128, 1)
    u = np.arange(TW, dtype=np.float64).reshape(1, TW)
    dist = np.abs(i + 1920.0 - u)
    tabs = []
    for g in range(4):
        tg = np.empty([4, 128, TW], dtype=ml_dtypes.bfloat16)
        for hh in range(4):
            slope = 2.0 ** (-(4 * g + hh + 1))
            tg[hh] = np.exp(-slope * dist / 8.0).astype(ml_dtypes.bfloat16)
        tabs.append(tg)
    return tabs


def kernel(x, Wq, Wk, Wv, Wo, bo, _trace=False, _trace_kwargs=None):
    global _NC, LAST_RESULTS
    x = np.asarray(x, dtype=np.float32)
    Wq = np.asarray(Wq, dtype=np.float32)
    Wk = np.asarray(Wk, dtype=np.float32)
    Wv = np.asarray(Wv, dtype=np.float32)
    Wo = np.asarray(Wo, dtype=np.float32)
    bo = np.asarray(bo, dtype=np.float32)

    if _NC is None:
        _NC = _build()
    nc = _NC

    tabs = _texp_tables()
    bf = ml_dtypes.bfloat16
    bo4 = (bo * 0.25).reshape(1, E).astype(bf)
    in_maps = []
    for core in range(8):
        n, g = core // 4, core % 4
        hs = slice(4 * g * D, (4 * g + 4) * D)
        kvs = slice(2 * g * D, (2 * g + 2) * D)
        in_maps.append(
            {
                "xT": np.ascontiguousarray(x[n].T).astype(bf),
                "wqT": np.ascontiguousarray(Wq[hs].T).astype(bf),
                "wkT": np.ascontiguousarray(Wk[kvs].T).astype(bf),
                "wvT": np.ascontiguousarray(Wv[kvs].T).astype(bf),
                "woT": np.ascontiguousarray(Wo[:, hs].T).astype(bf),
                "bo4": bo4,
                "texp": tabs[g],
            }
        )

    kw = {}
    if _trace:
        kw["trace"] = True
        kw.update(_trace_kwargs or {})
    res = run_bass_kernel_spmd(nc, in_maps, list(range(8)), **kw)
    LAST_RESULTS = res

    out = np.empty((2, S, E), dtype=np.float32)
    for n in range(2):
        acc = res.results[n * 4]["outT"].astype(np.float32)
        for g in range(1, 4):
            acc = acc + res.results[n * 4 + g]["outT"]
        out[n] = acc.T
    return out


# revision 21
# speedup vs baseline: 1.0984x; 1.0984x over previous
"""MultiHeadAttention (GQA + symmetric ALiBi) on 8 trn2 NeuronCores.

Sharding: core = (batch n in {0,1}) x (head-group g in {0..3}).
Each core handles 4 query heads (one GQA pair of kv heads) for one batch:
  - QKV projections in fp32 (x^T streamed, weights column-sliced per group)
  - S^T = K^T.T @ Q^T per (head, k-chunk) in fp32
  - softmax: exp((S + alibi)/8) = exp(S/8) * exp(alibi/8); the alibi factor
    is a host-precomputed bf16 table indexed by (i - u + 1920); no max-sub
    needed (exp args are O(1))
  - row sums via a ones-column appended to V (bf16 PV matmul, fp32 accum)
  - normalization: reciprocal of sums broadcast across partitions via a
    rank-1 matmul with a ones vector
  - output projection partial = attnT @ Wo^T slice + bo/4 (bias folded in as
    a K=1 matmul); host sums the 4 group partials per batch.
"""
import sys

sys.path.insert(0, "/opt/trn_rl_repo")
import numpy as np
import ml_dtypes

import concourse.bass as bass
import concourse.mybir as mybir
from concourse import bacc
from concourse.tile import TileContext
from concourse.masks import make_identity
from concourse.bass_utils import run_bass_kernel_spmd


def _register_ntff_hook_module():
    # bass_utils imports antenv.axon_hooks for trace=True under axon; this
    # image's antenv lacks it, so register our shim in sys.modules.
    import importlib.util

    if "antenv.axon_hooks" in sys.modules:
        return
    path = "/opt/trn_rl_repo/antenv/axon_hooks.py"
    try:
        spec = importlib.util.spec_from_file_location("antenv.axon_hooks", path)
        mod = importlib.util.module_from_spec(spec)
        spec.loader.exec_module(mod)
        sys.modules["antenv.axon_hooks"] = mod
    except Exception:
        pass


_register_ntff_hook_module()

S = 2048
E = 1024
D = 64
TW = 3968  # alibi exp-table width: u = j - k0 + 1920 in [0, 3968)
F32 = mybir.dt.float32
BF16 = mybir.dt.bfloat16

_NC = None
LAST_RESULTS = None


def _build():
    nc = bacc.Bacc("TRN2", target_bir_lowering=False, debug=False, num_devices=8)
    xT = nc.dram_tensor("xT", [E, S], BF16, kind="ExternalInput")
    wqT = nc.dram_tensor("wqT", [E, 256], BF16, kind="ExternalInput")
    wkT = nc.dram_tensor("wkT", [E, 128], BF16, kind="ExternalInput")
    wvT = nc.dram_tensor("wvT", [E, 128], BF16, kind="ExternalInput")
    woT = nc.dram_tensor("woT", [256, E], BF16, kind="ExternalInput")
    bo4 = nc.dram_tensor("bo4", [1, E], BF16, kind="ExternalInput")
    texp = nc.dram_tensor("texp", [4, 128, TW], BF16, kind="ExternalInput")
    outT = nc.dram_tensor("outT", [E, S], F32, kind="ExternalOutput")

    Exp = mybir.ActivationFunctionType.Exp

    with TileContext(nc) as tc:
        with (
            tc.sbuf_pool(name="const", bufs=1) as const,
            tc.sbuf_pool(name="qkv", bufs=1) as qkv,
            tc.sbuf_pool(name="pp", bufs=4) as pp,
            tc.sbuf_pool(name="norm", bufs=1) as norm,
        ):
            # ---- constants / weights
            wq_sb = const.tile([128, 8 * 256], BF16)
            nc.sync.dma_start(
                out=wq_sb.rearrange("p (c m) -> p c m", m=256),
                in_=wqT.rearrange("(c p) m -> p c m", p=128),
            )
            wk_sb = const.tile([128, 8 * 128], BF16)
            nc.sync.dma_start(
                out=wk_sb.rearrange("p (c m) -> p c m", m=128),
                in_=wkT.rearrange("(c p) m -> p c m", p=128),
            )
            wv_sb = const.tile([128, 8 * 128], BF16)
            nc.sync.dma_start(
                out=wv_sb.rearrange("p (c m) -> p c m", m=128),
                in_=wvT.rearrange("(c p) m -> p c m", p=128),
            )
            wo_sb = const.tile([128, 2 * 1024], BF16)
            nc.sync.dma_start(
                out=wo_sb.rearrange("p (c m) -> p c m", m=1024),
                in_=woT.rearrange("(c p) m -> p c m", p=128),
            )
            bo_sb = const.tile([1, E], BF16)
            nc.sync.dma_start(out=bo_sb, in_=bo4[:, :])
            tex_sb = const.tile([128, 4 * TW], BF16)
            for h in range(4):
                nc.sync.dma_start(out=tex_sb[:, h * TW : (h + 1) * TW], in_=texp[h])
            ones_sb = const.tile([1, 512], BF16)
            nc.vector.memset(ones_sb, 1.0)
            ident = const.tile([128, 128], BF16)
            make_identity(nc, ident)

            # QT/KT zero-padded to K=128 and VS to 128 cols: square bf16
            # stationaries keep the PE weight path fast (FWL), and the zero
            # rows/cols contribute nothing to the results.
            QT = [qkv.tile([128, S], BF16, name=f"qt{h}") for h in range(4)]
            KT = [qkv.tile([128, S], BF16, name=f"kt{k}") for k in range(2)]
            vt_sb = qkv.tile([128, S], BF16)
            VS = [qkv.tile([128, 16 * 128], BF16, name=f"vs{k}") for k in range(2)]
            AT = [qkv.tile([128, S], BF16, name=f"at{c}") for c in range(2)]
            for t in QT + KT:
                nc.vector.memset(t, 0.0)

            # ---- Phase A: QKV projections (fp32)
            with (
                tc.sbuf_pool(name="xp", bufs=3) as xp,
                tc.psum_pool(name="qkvps", bufs=4) as qp,
            ):
                for qh in range(2):  # 1024-wide q halves
                    ps_q0 = qp.tile([128, 1024], F32, tag="qkvps", name="ps_q0")
                    ps_q1 = qp.tile([128, 1024], F32, tag="qkvps", name="ps_q1")
                    ps_k = qp.tile([128, 1024], F32, tag="qkvps", name="ps_k")
                    ps_v = qp.tile([128, 1024], F32, tag="qkvps", name="ps_v")
                    for e in range(8):
                        xt = xp.tile([128, 1024], BF16, tag="x", name="xt")
                        nc.sync.dma_start(
                            out=xt,
                            in_=xT[e * 128 : (e + 1) * 128, qh * 1024 : (qh + 1) * 1024],
                        )
                        st, sp = (e == 0), (e == 7)
                        targets = [
                            (ps_q0, wq_sb[:, e * 256 : e * 256 + 128]),
                            (ps_q1, wq_sb[:, e * 256 + 128 : e * 256 + 256]),
                            (ps_k, wk_sb[:, e * 128 : (e + 1) * 128]),
                            (ps_v, wv_sb[:, e * 128 : (e + 1) * 128]),
                        ]
                        for ps, w in targets:
                            for i in range(2):
                                osl = slice(i * 512, (i + 1) * 512)
                                nc.tensor.matmul(
                                    ps[:, osl], w, xt[:, osl], start=st, stop=sp,
                                )
                    qs = slice(qh * 1024, (qh + 1) * 1024)
                    nc.vector.tensor_copy(vt_sb[:, qs], ps_v)
                    nc.vector.tensor_copy(KT[0][0:64, qs], ps_k[0:64, :])
                    nc.vector.tensor_copy(KT[1][0:64, qs], ps_k[64:128, :])
                    nc.scalar.copy(QT[0][0:64, qs], ps_q0[0:64, :])
                    nc.scalar.copy(QT[1][0:64, qs], ps_q0[64:128, :])
                    nc.scalar.copy(QT[2][0:64, qs], ps_q1[0:64, :])
                    nc.scalar.copy(QT[3][0:64, qs], ps_q1[64:128, :])

            # ---- Phase A2: V^T -> V (PE transposes), append ones column
            for kv in range(2):
                nc.vector.memset(VS[kv], 0.0)
                nc.vector.memset(
                    VS[kv].rearrange("p (c m) -> p c m", m=128)[:, :, 64:65], 1.0
                )
            with tc.psum_pool(name="tpps", bufs=4) as tp:
                for kc in range(16):
                    pt = tp.tile([128, 128], BF16, tag="tp", name="pt")
                    nc.tensor.transpose(pt, vt_sb[:, kc * 128 : (kc + 1) * 128], ident)
                    for kv in range(2):
                        nc.vector.tensor_copy(
                            VS[kv][:, kc * 128 : kc * 128 + 64],
                            pt[:, kv * 64 : (kv + 1) * 64],
                        )

            # ---- Phase C: attention per head
            with (
                tc.psum_pool(name="sps", bufs=2) as spp,
                tc.psum_pool(name="pvps", bufs=1) as pvp,
            ):
                pend = {}  # h -> (pvs, r) awaiting broadcast + final scale

                def head_attention(h):
                    kv = h // 2
                    pv = pvp.tile([128, 2048], F32, tag="pv", name="pv")
                    for kc in range(16):
                        ks = slice(kc * 128, (kc + 1) * 128)
                        # 4 S matmuls back-to-back (one stationary load)
                        ss = [
                            spp.tile([128, 1024], F32, tag="s", name="s"),
                            spp.tile([128, 1024], F32, tag="s", name="s2"),
                        ]
                        for qq in range(4):
                            nc.tensor.matmul(
                                ss[qq // 2][:, (qq % 2) * 512 : (qq % 2 + 1) * 512],
                                KT[kv][:, ks],
                                QT[h][:, qq * 512 : (qq + 1) * 512],
                                start=True, stop=True,
                            )
                        ps = []
                        for qh in range(2):
                            pexp = pp.tile([128, 1024], BF16, tag="pexp", name="pexp")
                            nc.scalar.activation(pexp, ss[qh], Exp, scale=0.125)
                            p = pp.tile([128, 1024], BF16, tag="p", name="p")
                            u0 = h * TW + 1920 - kc * 128 + qh * 1024
                            nc.vector.tensor_mul(p, pexp, tex_sb[:, u0 : u0 + 1024])
                            ps.append(p)
                        # 4 PV matmuls back-to-back (one stationary load)
                        for qq in range(4):
                            nc.tensor.matmul(
                                pv[:, qq * 512 : (qq + 1) * 512],
                                VS[kv][:, kc * 128 : (kc + 1) * 128],
                                ps[qq // 2][:, (qq % 2) * 512 : (qq % 2 + 1) * 512],
                                start=(kc == 0), stop=(kc == 15),
                                skip_group_check=True,
                            )
                    # Move pv out of PSUM at once (DVE + ACT in parallel) and
                    # run the reciprocal chain; the PE broadcast + final scale
                    # are deferred one head so this chain never stalls the PE.
                    pvs = norm.tile([64, 2048], F32, tag="pvs", name="pvs", bufs=2)
                    nc.vector.tensor_copy(pvs, pv[0:64, :])
                    rsum = norm.tile([1, 2048], F32, tag="rsum", name="rsum", bufs=2)
                    nc.scalar.copy(rsum, pv[64:65, :])
                    r128 = norm.tile([128, 16], F32, tag="r128", name="r128", bufs=2)
                    nc.sync.dma_start(out=r128, in_=rsum)
                    rr = norm.tile([128, 16], F32, tag="rr", name="rr", bufs=2)
                    nc.vector.reciprocal(rr, r128)
                    rrb = norm.tile([128, 16], BF16, tag="rrb", name="rrb", bufs=2)
                    nc.vector.tensor_copy(rrb, rr)
                    r = norm.tile([1, 2048], BF16, tag="r", name="r", bufs=2)
                    nc.sync.dma_start(out=r, in_=rrb)
                    pend[h] = (pvs, r)

                def head_normalize(h):
                    pvs, r = pend.pop(h)
                    rbs = norm.tile([64, 2048], F32, tag="rbs", name="rbs", bufs=1)
                    for half in range(2):
                        rb = spp.tile([64, 1024], F32, tag="s", name="rb")
                        for qq in range(2):
                            c0 = half * 1024 + qq * 512
                            nc.tensor.matmul(
                                rb[:, qq * 512 : (qq + 1) * 512],
                                (ones_sb[:, 0:64]),
                                (r[:, c0 : c0 + 512]),
                                start=True, stop=True,
                            )
                        nc.vector.tensor_copy(
                            rbs[:, half * 1024 : (half + 1) * 1024], rb
                        )
                    at = AT[h // 2]
                    r0 = 64 * (h % 2)
                    nc.vector.tensor_mul(at[r0 : r0 + 64, :], pvs, rbs)

                for h in range(4):
                    head_attention(h)
                    if h > 0:
                        head_normalize(h - 1)
                head_normalize(3)

            # ---- Phase D: output projection (+ bias/4 via K=1 matmul)
            with (
                tc.psum_pool(name="ops", bufs=8) as op,
                tc.sbuf_pool(name="osb", bufs=4) as osb,
            ):
                for ec in range(8):
                    os_ = [op.tile([128, 512], F32, tag="o", name="o") for _ in range(4)]
                    for c in range(2):
                        w = wo_sb[:, c * 1024 + ec * 128 : c * 1024 + (ec + 1) * 128]
                        for qc in range(4):
                            qs = slice(qc * 512, (qc + 1) * 512)
                            nc.tensor.matmul(
                                os_[qc], w, AT[c][:, qs],
                                start=(c == 0), stop=False,
                                skip_group_check=True,
                            )
                    wb = bo_sb[:, ec * 128 : (ec + 1) * 128]
                    for qc in range(4):
                        nc.tensor.matmul(
                            os_[qc], wb, ones_sb[:, 0:512],
                            start=False, stop=True,
                            skip_group_check=True,
                        )
                    for qc in range(4):
                        qs = slice(qc * 512, (qc + 1) * 512)
                        o_sb = osb.tile([128, 512], F32, tag="osb", name="o_sb")
                        nc.vector.tensor_copy(o_sb, os_[qc])
                        nc.sync.dma_start(
                            out=outT[ec * 128 : (ec + 1) * 128, qs], in_=o_sb
                        )

    nc.compile()
    return nc


def _texp_tables():
    i = np.arange(128, dtype=np.float64).reshape(128, 1)
    u = np.arange(TW, dtype=np.float64).reshape(1, TW)
    dist = np.abs(i + 1920.0 - u)
    tabs = []
    for g in range(4):
        tg = np.empty([4, 128, TW], dtype=ml_dtypes.bfloat16)
        for hh in range(4):
            slope = 2.0 ** (-(4 * g + hh + 1))
            tg[hh] = np.exp(-slope * dist / 8.0).astype(ml_dtypes.bfloat16)
        tabs.append(tg)
    return tabs


def kernel(x, Wq, Wk, Wv, Wo, bo, _trace=False, _trace_kwargs=None):
    global _NC, LAST_RESULTS
    x = np.asarray(x, dtype=np.float32)
    Wq = np.asarray(Wq, dtype=np.float32)
    Wk = np.asarray(Wk, dtype=np.float32)
    Wv = np.asarray(Wv, dtype=np.float32)
    Wo = np.asarray(Wo, dtype=np.float32)
    bo = np.asarray(bo, dtype=np.float32)

    if _NC is None:
        _NC = _build()
    nc = _NC

    tabs = _texp_tables()
    bf = ml_dtypes.bfloat16
    bo4 = (bo * 0.25).reshape(1, E).astype(bf)
    in_maps = []
    for core in range(8):
        n, g = core // 4, core % 4
        hs = slice(4 * g * D, (4 * g + 4) * D)
        kvs = slice(2 * g * D, (2 * g + 2) * D)
        in_maps.append(
            {
                "xT": np.ascontiguousarray(x[n].T).astype(bf),
                "wqT": np.ascontiguousarray(Wq[hs].T).astype(bf),
                "wkT": np.ascontiguousarray(Wk[kvs].T).astype(bf),
                "wvT": np.ascontiguousarray(Wv[kvs].T).astype(bf),
                "woT": np.ascontiguousarray(Wo[:, hs].T).astype(bf),
                "bo4": bo4,
                "texp": tabs[g],
            }
        )

    kw = {}
    if _trace:
        kw["trace"] = True
        kw.update(_trace_kwargs or {})
    res = run_bass_kernel_spmd(nc, in_maps, list(range(8)), **kw)
    LAST_RESULTS = res

    out = np.empty((2, S, E), dtype=np.float32)
    for n in range(2):
        acc = res.results[n * 4]["outT"].astype(np.float32)
        for g in range(1, 4):
            acc = acc + res.results[n * 4 + g]["outT"]
        out[n] = acc.T
    return out


# revision 22
# speedup vs baseline: 1.1419x; 1.0396x over previous
"""MultiHeadAttention (GQA + symmetric ALiBi) on 8 trn2 NeuronCores.

Sharding: core = (batch n in {0,1}) x (head-group g in {0..3}).
Each core handles 4 query heads (one GQA pair of kv heads) for one batch:
  - QKV projections in fp32 (x^T streamed, weights column-sliced per group)
  - S^T = K^T.T @ Q^T per (head, k-chunk) in fp32
  - softmax: exp((S + alibi)/8) = exp(S/8) * exp(alibi/8); the alibi factor
    is a host-precomputed bf16 table indexed by (i - u + 1920); no max-sub
    needed (exp args are O(1))
  - row sums via a ones-column appended to V (bf16 PV matmul, fp32 accum)
  - normalization: reciprocal of sums broadcast across partitions via a
    rank-1 matmul with a ones vector
  - output projection partial = attnT @ Wo^T slice + bo/4 (bias folded in as
    a K=1 matmul); host sums the 4 group partials per batch.
"""
import sys

sys.path.insert(0, "/opt/trn_rl_repo")
import numpy as np
import ml_dtypes

import concourse.bass as bass
import concourse.mybir as mybir
from concourse import bacc
from concourse.tile import TileContext
from concourse.masks import make_identity
from concourse.bass_utils import run_bass_kernel_spmd


def _register_ntff_hook_module():
    # bass_utils imports antenv.axon_hooks for trace=True under axon; this
    # image's antenv lacks it, so register our shim in sys.modules.
    import importlib.util

    if "antenv.axon_hooks" in sys.modules:
        return
    path = "/opt/trn_rl_repo/antenv/axon_hooks.py"
    try:
        spec = importlib.util.spec_from_file_location("antenv.axon_hooks", path)
        mod = importlib.util.module_from_spec(spec)
        spec.loader.exec_module(mod)
        sys.modules["antenv.axon_hooks"] = mod
    except Exception:
        pass


_register_ntff_hook_module()

S = 2048
E = 1024
D = 64
TW = 3968  # alibi exp-table width: u = j - k0 + 1920 in [0, 3968)
F32 = mybir.dt.float32
BF16 = mybir.dt.bfloat16

_NC = None
LAST_RESULTS = None


def _build():
    nc = bacc.Bacc("TRN2", target_bir_lowering=False, debug=False, num_devices=8)
    xT = nc.dram_tensor("xT", [E, S], BF16, kind="ExternalInput")
    wqT = nc.dram_tensor("wqT", [E, 256], BF16, kind="ExternalInput")
    wkT = nc.dram_tensor("wkT", [E, 128], BF16, kind="ExternalInput")
    wvT = nc.dram_tensor("wvT", [E, 128], BF16, kind="ExternalInput")
    woT = nc.dram_tensor("woT", [256, E], BF16, kind="ExternalInput")
    bo4 = nc.dram_tensor("bo4", [1, E], BF16, kind="ExternalInput")
    texp = nc.dram_tensor("texp", [4, 128, TW], BF16, kind="ExternalInput")
    outT = nc.dram_tensor("outT", [E, S], F32, kind="ExternalOutput")

    Exp = mybir.ActivationFunctionType.Exp

    with TileContext(nc) as tc:
        with (
            tc.sbuf_pool(name="const", bufs=1) as const,
            tc.sbuf_pool(name="qkv", bufs=1) as qkv,
            tc.sbuf_pool(name="pp", bufs=4) as pp,
            tc.sbuf_pool(name="norm", bufs=1) as norm,
        ):
            # ---- constants / weights
            wq_sb = const.tile([128, 8 * 256], BF16)
            nc.sync.dma_start(
                out=wq_sb.rearrange("p (c m) -> p c m", m=256),
                in_=wqT.rearrange("(c p) m -> p c m", p=128),
            )
            wk_sb = const.tile([128, 8 * 128], BF16)
            nc.sync.dma_start(
                out=wk_sb.rearrange("p (c m) -> p c m", m=128),
                in_=wkT.rearrange("(c p) m -> p c m", p=128),
            )
            wv_sb = const.tile([128, 8 * 128], BF16)
            nc.sync.dma_start(
                out=wv_sb.rearrange("p (c m) -> p c m", m=128),
                in_=wvT.rearrange("(c p) m -> p c m", p=128),
            )
            wo_sb = const.tile([128, 2 * 1024], BF16)
            nc.sync.dma_start(
                out=wo_sb.rearrange("p (c m) -> p c m", m=1024),
                in_=woT.rearrange("(c p) m -> p c m", p=128),
            )
            bo_sb = const.tile([1, E], BF16)
            nc.sync.dma_start(out=bo_sb, in_=bo4[:, :])
            tex_sb = const.tile([128, 4 * TW], BF16)
            for h in range(4):
                nc.sync.dma_start(out=tex_sb[:, h * TW : (h + 1) * TW], in_=texp[h])
            ones_sb = const.tile([1, 512], BF16)
            nc.vector.memset(ones_sb, 1.0)
            ident = const.tile([128, 128], BF16)
            make_identity(nc, ident)

            # QT/KT zero-padded to K=128 and VS to 128 cols: square bf16
            # stationaries keep the PE weight path fast (FWL), and the zero
            # rows/cols contribute nothing to the results.
            QT = [qkv.tile([128, S], BF16, name=f"qt{h}") for h in range(4)]
            KT = [qkv.tile([128, S], BF16, name=f"kt{k}") for k in range(2)]
            vt_sb = qkv.tile([128, S], BF16)
            VS = [qkv.tile([128, 16 * 128], BF16, name=f"vs{k}") for k in range(2)]
            AT = [qkv.tile([128, S], BF16, name=f"at{c}") for c in range(2)]
            for t in QT + KT:
                nc.vector.memset(t, 0.0)

            # ---- Phase A: QKV projections (fp32)
            with (
                tc.sbuf_pool(name="xp", bufs=3) as xp,
                tc.psum_pool(name="qkvps", bufs=4) as qp,
            ):
                for qh in range(2):  # 1024-wide q halves
                    ps_q0 = qp.tile([128, 1024], F32, tag="qkvps", name="ps_q0")
                    ps_q1 = qp.tile([128, 1024], F32, tag="qkvps", name="ps_q1")
                    ps_k = qp.tile([128, 1024], F32, tag="qkvps", name="ps_k")
                    ps_v = qp.tile([128, 1024], F32, tag="qkvps", name="ps_v")
                    for e in range(8):
                        xt = xp.tile([128, 1024], BF16, tag="x", name="xt")
                        nc.sync.dma_start(
                            out=xt,
                            in_=xT[e * 128 : (e + 1) * 128, qh * 1024 : (qh + 1) * 1024],
                        )
                        st, sp = (e == 0), (e == 7)
                        targets = [
                            (ps_q0, wq_sb[:, e * 256 : e * 256 + 128]),
                            (ps_q1, wq_sb[:, e * 256 + 128 : e * 256 + 256]),
                            (ps_k, wk_sb[:, e * 128 : (e + 1) * 128]),
                            (ps_v, wv_sb[:, e * 128 : (e + 1) * 128]),
                        ]
                        for ps, w in targets:
                            for i in range(2):
                                osl = slice(i * 512, (i + 1) * 512)
                                nc.tensor.matmul(
                                    ps[:, osl], w, xt[:, osl], start=st, stop=sp,
                                )
                    qs = slice(qh * 1024, (qh + 1) * 1024)
                    nc.vector.tensor_copy(vt_sb[:, qs], ps_v)
                    nc.vector.tensor_copy(KT[0][0:64, qs], ps_k[0:64, :])
                    nc.vector.tensor_copy(KT[1][0:64, qs], ps_k[64:128, :])
                    nc.scalar.copy(QT[0][0:64, qs], ps_q0[0:64, :])
                    nc.scalar.copy(QT[1][0:64, qs], ps_q0[64:128, :])
                    nc.scalar.copy(QT[2][0:64, qs], ps_q1[0:64, :])
                    nc.scalar.copy(QT[3][0:64, qs], ps_q1[64:128, :])

            # ---- Phase A2: V^T -> V (PE transposes), append ones column
            for kv in range(2):
                nc.vector.memset(VS[kv], 0.0)
                nc.vector.memset(
                    VS[kv].rearrange("p (c m) -> p c m", m=128)[:, :, 64:65], 1.0
                )
            with tc.psum_pool(name="tpps", bufs=4) as tp:
                for kc in range(16):
                    pt = tp.tile([128, 128], BF16, tag="tp", name="pt")
                    nc.tensor.transpose(pt, vt_sb[:, kc * 128 : (kc + 1) * 128], ident)
                    for kv in range(2):
                        nc.vector.tensor_copy(
                            VS[kv][:, kc * 128 : kc * 128 + 64],
                            pt[:, kv * 64 : (kv + 1) * 64],
                        )

            # ---- Phase C: attention per head
            with (
                tc.psum_pool(name="sps", bufs=2) as spp,
                tc.psum_pool(name="pvps", bufs=1) as pvp,
            ):
                pend = {}  # h -> (pvs, r) awaiting broadcast + final scale

                def head_attention(h):
                    kv = h // 2
                    pv = pvp.tile([128, 2048], F32, tag="pv", name="pv")
                    for kc in range(16):
                        ks = slice(kc * 128, (kc + 1) * 128)
                        # 4 S matmuls back-to-back (one stationary load)
                        ss = [
                            spp.tile([128, 1024], F32, tag="s", name="s"),
                            spp.tile([128, 1024], F32, tag="s", name="s2"),
                        ]
                        for qq in range(4):
                            nc.tensor.matmul(
                                ss[qq // 2][:, (qq % 2) * 512 : (qq % 2 + 1) * 512],
                                KT[kv][:, ks],
                                QT[h][:, qq * 512 : (qq + 1) * 512],
                                start=True, stop=True,
                            )
                        ps = []
                        for qh in range(2):
                            pexp = pp.tile([128, 1024], BF16, tag="pexp", name="pexp")
                            nc.scalar.activation(pexp, ss[qh], Exp, scale=0.125)
                            p = pp.tile([128, 1024], BF16, tag="p", name="p")
                            u0 = h * TW + 1920 - kc * 128 + qh * 1024
                            nc.vector.tensor_mul(p, pexp, tex_sb[:, u0 : u0 + 1024])
                            ps.append(p)
                        # 4 PV matmuls back-to-back (one stationary load)
                        for qq in range(4):
                            nc.tensor.matmul(
                                pv[:, qq * 512 : (qq + 1) * 512],
                                VS[kv][:, kc * 128 : (kc + 1) * 128],
                                ps[qq // 2][:, (qq % 2) * 512 : (qq % 2 + 1) * 512],
                                start=(kc == 0), stop=(kc == 15),
                                skip_group_check=True,
                            )
                    # Move pv out of PSUM at once (DVE + ACT in parallel) and
                    # run the reciprocal chain; the PE broadcast + final scale
                    # are deferred one head so this chain never stalls the PE.
                    pvs = norm.tile([64, 2048], F32, tag="pvs", name="pvs", bufs=2)
                    nc.vector.tensor_copy(pvs, pv[0:64, :])
                    rsum = norm.tile([1, 2048], F32, tag="rsum", name="rsum", bufs=2)
                    nc.scalar.copy(rsum, pv[64:65, :])
                    r128 = norm.tile([128, 16], F32, tag="r128", name="r128", bufs=2)
                    nc.sync.dma_start(out=r128, in_=rsum)
                    rr = norm.tile([128, 16], F32, tag="rr", name="rr", bufs=2)
                    nc.vector.reciprocal(rr, r128)
                    rrb = norm.tile([128, 16], BF16, tag="rrb", name="rrb", bufs=2)
                    nc.vector.tensor_copy(rrb, rr)
                    r = norm.tile([1, 2048], BF16, tag="r", name="r", bufs=2)
                    nc.sync.dma_start(out=r, in_=rrb)
                    pend[h] = (pvs, r)

                def head_normalize(h):
                    pvs, r = pend.pop(h)
                    rbs = norm.tile([64, 2048], F32, tag="rbs", name="rbs", bufs=2)
                    for half in range(2):
                        rb = spp.tile([64, 1024], F32, tag="s", name="rb")
                        for qq in range(2):
                            c0 = half * 1024 + qq * 512
                            nc.tensor.matmul(
                                rb[:, qq * 512 : (qq + 1) * 512],
                                (ones_sb[:, 0:64]),
                                (r[:, c0 : c0 + 512]),
                                start=True, stop=True,
                            )
                        nc.vector.tensor_copy(
                            rbs[:, half * 1024 : (half + 1) * 1024], rb
                        )
                    at = AT[h // 2]
                    r0 = 64 * (h % 2)
                    nc.vector.tensor_mul(at[r0 : r0 + 64, :], pvs, rbs)

                for h in range(4):
                    head_attention(h)
                    if h > 0:
                        head_normalize(h - 1)
                head_normalize(3)

            # ---- Phase D: output projection (+ bias/4 via K=1 matmul)
            with (
                tc.psum_pool(name="ops", bufs=8) as op,
                tc.sbuf_pool(name="osb", bufs=4) as osb,
            ):
                for ec in range(8):
                    os_ = [op.tile([128, 512], F32, tag="o", name="o") for _ in range(4)]
                    for c in range(2):
                        w = wo_sb[:, c * 1024 + ec * 128 : c * 1024 + (ec + 1) * 128]
                        for qc in range(4):
                            qs = slice(qc * 512, (qc + 1) * 512)
                            nc.tensor.matmul(
                                os_[qc], w, AT[c][:, qs],
                                start=(c == 0), stop=False,
                                skip_group_check=True,
                            )
                    wb = bo_sb[:, ec * 128 : (ec + 1) * 128]
                    for qc in range(4):
                        nc.tensor.matmul(
                            os_[qc], wb, ones_sb[:, 0:512],
                            start=False, stop=True,
                            skip_group_check=True,
                        )
                    for qc in range(4):
                        qs = slice(qc * 512, (qc + 1) * 512)
                        o_sb = osb.tile([128, 512], F32, tag="osb", name="o_sb")
                        nc.vector.tensor_copy(o_sb, os_[qc])
                        nc.sync.dma_start(
                            out=outT[ec * 128 : (ec + 1) * 128, qs], in_=o_sb
                        )

    nc.compile()
    return nc


def _texp_tables():
    i = np.arange(128, dtype=np.float64).reshape(128, 1)
    u = np.arange(TW, dtype=np.float64).reshape(1, TW)
    dist = np.abs(i + 1920.0 - u)
    tabs = []
    for g in range(4):
        tg = np.empty([4, 128, TW], dtype=ml_dtypes.bfloat16)
        for hh in range(4):
            slope = 2.0 ** (-(4 * g + hh + 1))
            tg[hh] = np.exp(-slope * dist / 8.0).astype(ml_dtypes.bfloat16)
        tabs.append(tg)
    return tabs


def kernel(x, Wq, Wk, Wv, Wo, bo, _trace=False, _trace_kwargs=None):
    global _NC, LAST_RESULTS
    x = np.asarray(x, dtype=np.float32)
    Wq = np.asarray(Wq, dtype=np.float32)
    Wk = np.asarray(Wk, dtype=np.float32)
    Wv = np.asarray(Wv, dtype=np.float32)
    Wo = np.asarray(Wo, dtype=np.float32)
    bo = np.asarray(bo, dtype=np.float32)

    if _NC is None:
        _NC = _build()
    nc = _NC

    tabs = _texp_tables()
    bf = ml_dtypes.bfloat16
    bo4 = (bo * 0.25).reshape(1, E).astype(bf)
    in_maps = []
    for core in range(8):
        n, g = core // 4, core % 4
        hs = slice(4 * g * D, (4 * g + 4) * D)
        kvs = slice(2 * g * D, (2 * g + 2) * D)
        in_maps.append(
            {
                "xT": np.ascontiguousarray(x[n].T).astype(bf),
                "wqT": np.ascontiguousarray(Wq[hs].T).astype(bf),
                "wkT": np.ascontiguousarray(Wk[kvs].T).astype(bf),
                "wvT": np.ascontiguousarray(Wv[kvs].T).astype(bf),
                "woT": np.ascontiguousarray(Wo[:, hs].T).astype(bf),
                "bo4": bo4,
                "texp": tabs[g],
            }
        )

    kw = {}
    if _trace:
        kw["trace"] = True
        kw.update(_trace_kwargs or {})
    res = run_bass_kernel_spmd(nc, in_maps, list(range(8)), **kw)
    LAST_RESULTS = res

    out = np.empty((2, S, E), dtype=np.float32)
    for n in range(2):
        acc = res.results[n * 4]["outT"].astype(np.float32)
        for g in range(1, 4):
            acc = acc + res.results[n * 4 + g]["outT"]
        out[n] = acc.T
    return out


# revision 23
# speedup vs baseline: 1.1826x; 1.0356x over previous
"""MultiHeadAttention (GQA + symmetric ALiBi) on 8 trn2 NeuronCores.

Sharding: core = (batch n in {0,1}) x (head-group g in {0..3}).
Each core handles 4 query heads (one GQA pair of kv heads) for one batch:
  - QKV projections in fp32 (x^T streamed, weights column-sliced per group)
  - S^T = K^T.T @ Q^T per (head, k-chunk) in fp32
  - softmax: exp((S + alibi)/8) = exp(S/8) * exp(alibi/8); the alibi factor
    is a host-precomputed bf16 table indexed by (i - u + 1920); no max-sub
    needed (exp args are O(1))
  - row sums via a ones-column appended to V (bf16 PV matmul, fp32 accum)
  - normalization: reciprocal of sums broadcast across partitions via a
    rank-1 matmul with a ones vector
  - output projection partial = attnT @ Wo^T slice + bo/4 (bias folded in as
    a K=1 matmul); host sums the 4 group partials per batch.
"""
import sys

sys.path.insert(0, "/opt/trn_rl_repo")
import numpy as np
import ml_dtypes

import concourse.bass as bass
import concourse.mybir as mybir
from concourse import bacc
from concourse.tile import TileContext
from concourse.masks import make_identity
from concourse.bass_utils import run_bass_kernel_spmd


def _register_ntff_hook_module():
    # bass_utils imports antenv.axon_hooks for trace=True under axon; this
    # image's antenv lacks it, so register our shim in sys.modules.
    import importlib.util

    if "antenv.axon_hooks" in sys.modules:
        return
    path = "/opt/trn_rl_repo/antenv/axon_hooks.py"
    try:
        spec = importlib.util.spec_from_file_location("antenv.axon_hooks", path)
        mod = importlib.util.module_from_spec(spec)
        spec.loader.exec_module(mod)
        sys.modules["antenv.axon_hooks"] = mod
    except Exception:
        pass


_register_ntff_hook_module()

S = 2048
E = 1024
D = 64
TW = 3968  # alibi exp-table width: u = j - k0 + 1920 in [0, 3968)
F32 = mybir.dt.float32
BF16 = mybir.dt.bfloat16

_NC = None
LAST_RESULTS = None


def _build():
    nc = bacc.Bacc("TRN2", target_bir_lowering=False, debug=False, num_devices=8)
    xT = nc.dram_tensor("xT", [E, S], BF16, kind="ExternalInput")
    wqT = nc.dram_tensor("wqT", [E, 256], BF16, kind="ExternalInput")
    wkT = nc.dram_tensor("wkT", [E, 128], BF16, kind="ExternalInput")
    wvT = nc.dram_tensor("wvT", [E, 128], BF16, kind="ExternalInput")
    woT = nc.dram_tensor("woT", [256, E], BF16, kind="ExternalInput")
    bo4 = nc.dram_tensor("bo4", [128, 8], F32, kind="ExternalInput")
    texp = nc.dram_tensor("texp", [4, 128, TW], BF16, kind="ExternalInput")
    outT = nc.dram_tensor("outT", [E, S], F32, kind="ExternalOutput")

    Exp = mybir.ActivationFunctionType.Exp

    with TileContext(nc) as tc:
        with (
            tc.sbuf_pool(name="const", bufs=1) as const,
            tc.sbuf_pool(name="qkv", bufs=1) as qkv,
            tc.sbuf_pool(name="pp", bufs=4) as pp,
            tc.sbuf_pool(name="norm", bufs=1) as norm,
        ):
            # ---- constants / weights
            wq_sb = const.tile([128, 8 * 256], BF16)
            nc.sync.dma_start(
                out=wq_sb.rearrange("p (c m) -> p c m", m=256),
                in_=wqT.rearrange("(c p) m -> p c m", p=128),
            )
            wk_sb = const.tile([128, 8 * 128], BF16)
            nc.sync.dma_start(
                out=wk_sb.rearrange("p (c m) -> p c m", m=128),
                in_=wkT.rearrange("(c p) m -> p c m", p=128),
            )
            wv_sb = const.tile([128, 8 * 128], BF16)
            nc.sync.dma_start(
                out=wv_sb.rearrange("p (c m) -> p c m", m=128),
                in_=wvT.rearrange("(c p) m -> p c m", p=128),
            )
            wo_sb = const.tile([128, 2 * 1024], BF16)
            bo_sb = const.tile([128, 8], F32)
            tex_sb = const.tile([128, 4 * TW], BF16)
            ones_sb = const.tile([1, 512], BF16)
            nc.vector.memset(ones_sb, 1.0)
            ident = const.tile([128, 128], BF16)
            make_identity(nc, ident)

            # QT/KT zero-padded to K=128 and VS to 128 cols: square bf16
            # stationaries keep the PE weight path fast (FWL), and the zero
            # rows/cols contribute nothing to the results.
            QT = [qkv.tile([128, S], BF16, name=f"qt{h}") for h in range(4)]
            KT = [qkv.tile([128, S], BF16, name=f"kt{k}") for k in range(2)]
            vt_sb = qkv.tile([128, S], BF16)
            VS = [qkv.tile([128, 16 * 128], BF16, name=f"vs{k}") for k in range(2)]
            AT = [qkv.tile([128, S], BF16, name=f"at{c}") for c in range(2)]
            for t in QT + KT:
                nc.vector.memset(t, 0.0)

            # ---- Phase A: QKV projections (fp32)
            with (
                tc.sbuf_pool(name="xp", bufs=3) as xp,
                tc.psum_pool(name="qkvps", bufs=4) as qp,
            ):
                for qh in range(2):  # 1024-wide q halves
                    ps_q0 = qp.tile([128, 1024], F32, tag="qkvps", name="ps_q0")
                    ps_q1 = qp.tile([128, 1024], F32, tag="qkvps", name="ps_q1")
                    ps_k = qp.tile([128, 1024], F32, tag="qkvps", name="ps_k")
                    ps_v = qp.tile([128, 1024], F32, tag="qkvps", name="ps_v")
                    for e in range(8):
                        xt = xp.tile([128, 1024], BF16, tag="x", name="xt")
                        nc.sync.dma_start(
                            out=xt,
                            in_=xT[e * 128 : (e + 1) * 128, qh * 1024 : (qh + 1) * 1024],
                        )
                        st, sp = (e == 0), (e == 7)
                        targets = [
                            (ps_q0, wq_sb[:, e * 256 : e * 256 + 128]),
                            (ps_q1, wq_sb[:, e * 256 + 128 : e * 256 + 256]),
                            (ps_k, wk_sb[:, e * 128 : (e + 1) * 128]),
                            (ps_v, wv_sb[:, e * 128 : (e + 1) * 128]),
                        ]
                        for ps, w in targets:
                            for i in range(2):
                                osl = slice(i * 512, (i + 1) * 512)
                                nc.tensor.matmul(
                                    ps[:, osl], w, xt[:, osl], start=st, stop=sp,
                                )
                    qs = slice(qh * 1024, (qh + 1) * 1024)
                    nc.vector.tensor_copy(vt_sb[:, qs], ps_v)
                    nc.vector.tensor_copy(KT[0][0:64, qs], ps_k[0:64, :])
                    nc.vector.tensor_copy(KT[1][0:64, qs], ps_k[64:128, :])
                    nc.scalar.copy(QT[0][0:64, qs], ps_q0[0:64, :])
                    nc.scalar.copy(QT[1][0:64, qs], ps_q0[64:128, :])
                    nc.scalar.copy(QT[2][0:64, qs], ps_q1[0:64, :])
                    nc.scalar.copy(QT[3][0:64, qs], ps_q1[64:128, :])

            # late-need constants: emitted after phase A so their DMAs don't
            # delay the x tiles at startup
            nc.sync.dma_start(
                out=wo_sb.rearrange("p (c m) -> p c m", m=1024),
                in_=woT.rearrange("(c p) m -> p c m", p=128),
            )
            nc.sync.dma_start(out=bo_sb, in_=bo4[:, :])
            for h in range(4):
                nc.sync.dma_start(out=tex_sb[:, h * TW : (h + 1) * TW], in_=texp[h])

            # ---- Phase A2: V^T -> V (PE transposes), append ones column
            for kv in range(2):
                nc.vector.memset(VS[kv], 0.0)
                nc.vector.memset(
                    VS[kv].rearrange("p (c m) -> p c m", m=128)[:, :, 64:65], 1.0
                )
            with tc.psum_pool(name="tpps", bufs=4) as tp:
                for kc in range(16):
                    pt = tp.tile([128, 128], BF16, tag="tp", name="pt")
                    nc.tensor.transpose(pt, vt_sb[:, kc * 128 : (kc + 1) * 128], ident)
                    for kv in range(2):
                        nc.vector.tensor_copy(
                            VS[kv][:, kc * 128 : kc * 128 + 64],
                            pt[:, kv * 64 : (kv + 1) * 64],
                        )

            # ---- Phase C: attention per head
            with (
                tc.psum_pool(name="sps", bufs=2) as spp,
                tc.psum_pool(name="pvps", bufs=1) as pvp,
            ):
                pend = {}  # h -> (pvs, r) awaiting broadcast + final scale

                def head_attention(h):
                    kv = h // 2
                    pv = pvp.tile([128, 2048], F32, tag="pv", name="pv")
                    for kc in range(16):
                        ks = slice(kc * 128, (kc + 1) * 128)
                        # 4 S matmuls back-to-back (one stationary load)
                        ss = [
                            spp.tile([128, 1024], F32, tag="s", name="s"),
                            spp.tile([128, 1024], F32, tag="s", name="s2"),
                        ]
                        for qq in range(4):
                            nc.tensor.matmul(
                                ss[qq // 2][:, (qq % 2) * 512 : (qq % 2 + 1) * 512],
                                KT[kv][:, ks],
                                QT[h][:, qq * 512 : (qq + 1) * 512],
                                start=True, stop=True,
                            )
                        ps = []
                        for qh in range(2):
                            pexp = pp.tile([128, 1024], BF16, tag="pexp", name="pexp")
                            nc.scalar.activation(pexp, ss[qh], Exp, scale=0.125)
                            p = pp.tile([128, 1024], BF16, tag="p", name="p")
                            u0 = h * TW + 1920 - kc * 128 + qh * 1024
                            nc.vector.tensor_mul(p, pexp, tex_sb[:, u0 : u0 + 1024])
                            ps.append(p)
                        # 4 PV matmuls back-to-back (one stationary load)
                        for qq in range(4):
                            nc.tensor.matmul(
                                pv[:, qq * 512 : (qq + 1) * 512],
                                VS[kv][:, kc * 128 : (kc + 1) * 128],
                                ps[qq // 2][:, (qq % 2) * 512 : (qq % 2 + 1) * 512],
                                start=(kc == 0), stop=(kc == 15),
                                skip_group_check=True,
                            )
                    # Move pv out of PSUM at once (DVE + ACT in parallel) and
                    # run the reciprocal chain; the PE broadcast + final scale
                    # are deferred one head so this chain never stalls the PE.
                    pvs = norm.tile([64, 2048], F32, tag="pvs", name="pvs", bufs=2)
                    nc.vector.tensor_copy(pvs, pv[0:64, :])
                    rsum = norm.tile([1, 2048], F32, tag="rsum", name="rsum", bufs=2)
                    nc.scalar.copy(rsum, pv[64:65, :])
                    r128 = norm.tile([128, 16], F32, tag="r128", name="r128", bufs=2)
                    nc.sync.dma_start(out=r128, in_=rsum)
                    rr = norm.tile([128, 16], F32, tag="rr", name="rr", bufs=2)
                    nc.vector.reciprocal(rr, r128)
                    rrb = norm.tile([128, 16], BF16, tag="rrb", name="rrb", bufs=2)
                    nc.vector.tensor_copy(rrb, rr)
                    r = norm.tile([1, 2048], BF16, tag="r", name="r", bufs=2)
                    nc.sync.dma_start(out=r, in_=rrb)
                    pend[h] = (pvs, r)

                def head_normalize(h):
                    pvs, r = pend.pop(h)
                    rbs = norm.tile([64, 2048], F32, tag="rbs", name="rbs", bufs=2)
                    for half in range(2):
                        rb = spp.tile([64, 1024], F32, tag="s", name="rb")
                        for qq in range(2):
                            c0 = half * 1024 + qq * 512
                            nc.tensor.matmul(
                                rb[:, qq * 512 : (qq + 1) * 512],
                                (ones_sb[:, 0:64]),
                                (r[:, c0 : c0 + 512]),
                                start=True, stop=True,
                            )
                        nc.vector.tensor_copy(
                            rbs[:, half * 1024 : (half + 1) * 1024], rb
                        )
                    at = AT[h // 2]
                    r0 = 64 * (h % 2)
                    nc.vector.tensor_mul(at[r0 : r0 + 64, :], pvs, rbs)

                for h in range(4):
                    head_attention(h)
                    if h > 0:
                        head_normalize(h - 1)
                head_normalize(3)

            # ---- Phase D: output projection (+ bias/4 via K=1 matmul)
            with (
                tc.psum_pool(name="ops", bufs=8) as op,
                tc.sbuf_pool(name="osb", bufs=4) as osb,
            ):
                for ec in range(8):
                    os_ = [op.tile([128, 512], F32, tag="o", name="o") for _ in range(4)]
                    for c in range(2):
                        w = wo_sb[:, c * 1024 + ec * 128 : c * 1024 + (ec + 1) * 128]
                        for qc in range(4):
                            qs = slice(qc * 512, (qc + 1) * 512)
                            nc.tensor.matmul(
                                os_[qc], w, AT[c][:, qs],
                                start=(c == 0), stop=(c == 1),
                                skip_group_check=True,
                            )
                    for qc in range(4):
                        qs = slice(qc * 512, (qc + 1) * 512)
                        o_sb = osb.tile([128, 512], F32, tag="osb", name="o_sb")
                        nc.vector.tensor_scalar_add(
                            o_sb, os_[qc], bo_sb[:, ec : ec + 1]
                        )
                        nc.sync.dma_start(
                            out=outT[ec * 128 : (ec + 1) * 128, qs], in_=o_sb
                        )

    nc.compile()
    return nc


def _texp_tables():
    i = np.arange(128, dtype=np.float64).reshape(128, 1)
    u = np.arange(TW, dtype=np.float64).reshape(1, TW)
    dist = np.abs(i + 1920.0 - u)
    tabs = []
    for g in range(4):
        tg = np.empty([4, 128, TW], dtype=ml_dtypes.bfloat16)
        for hh in range(4):
            slope = 2.0 ** (-(4 * g + hh + 1))
            tg[hh] = np.exp(-slope * dist / 8.0).astype(ml_dtypes.bfloat16)
        tabs.append(tg)
    return tabs


def kernel(x, Wq, Wk, Wv, Wo, bo, _trace=False, _trace_kwargs=None):
    global _NC, LAST_RESULTS
    x = np.asarray(x, dtype=np.float32)
    Wq = np.asarray(Wq, dtype=np.float32)
    Wk = np.asarray(Wk, dtype=np.float32)
    Wv = np.asarray(Wv, dtype=np.float32)
    Wo = np.asarray(Wo, dtype=np.float32)
    bo = np.asarray(bo, dtype=np.float32)

    if _NC is None:
        _NC = _build()
    nc = _NC

    tabs = _texp_tables()
    bf = ml_dtypes.bfloat16
    bo4 = np.ascontiguousarray((bo * 0.25).reshape(8, 128).T).astype(np.float32)
    in_maps = []
    for core in range(8):
        n, g = core // 4, core % 4
        hs = slice(4 * g * D, (4 * g + 4) * D)
        kvs = slice(2 * g * D, (2 * g + 2) * D)
        in_maps.append(
            {
                "xT": np.ascontiguousarray(x[n].T).astype(bf),
                "wqT": np.ascontiguousarray(Wq[hs].T).astype(bf),
                "wkT": np.ascontiguousarray(Wk[kvs].T).astype(bf),
                "wvT": np.ascontiguousarray(Wv[kvs].T).astype(bf),
                "woT": np.ascontiguousarray(Wo[:, hs].T).astype(bf),
                "bo4": bo4,
                "texp": tabs[g],
            }
        )

    kw = {}
    if _trace:
        kw["trace"] = True
        kw.update(_trace_kwargs or {})
    res = run_bass_kernel_spmd(nc, in_maps, list(range(8)), **kw)
    LAST_RESULTS = res

    out = np.empty((2, S, E), dtype=np.float32)
    for n in range(2):
        acc = res.results[n * 4]["outT"].astype(np.float32)
        for g in range(1, 4):
            acc = acc + res.results[n * 4 + g]["outT"]
        out[n] = acc.T
    return out
